# revision 4
# baseline (speedup 1.0000x reference)
"""Trainium2 Bass kernel: disparity regression via top-2 over the last axis.

pred[b, n] = sum_k topi_k * softmax(topv_k)  with K=2 over cost[b, n, :192].

Implementation: flatten to [524288, 192] rows, shard 8 ways by row across the
NeuronCores. Per core, super-tiles of [128 partitions x 64 rows x 192] stream
through DVE max8 (top-8 values, descending) + max_index (their positions,
lowest-index-first on ties, matching jax.lax.top_k), then one batched epilogue
per super-tile: d = v2 - v1, s = sigmoid(d) on ScalarE, pred = i1 + (i2-i1)*s.
Selection is exact fp32, so ordering is bit-identical to the reference.
"""
import numpy as np

import concourse.bacc as bacc
import concourse.tile as tile
import concourse.mybir as mybir
from concourse.bass_utils import run_bass_kernel_spmd

N_CORES = 8
B, N, D = 4, 131072, 192
ROWS = B * N                       # 524288
ROWS_PER_CORE = ROWS // N_CORES    # 65536
P = 128                            # SBUF partitions
G = 64                             # rows per partition per super-tile
TILE_ROWS = P * G                  # 8192
N_TILES = ROWS_PER_CORE // TILE_ROWS  # 8
QD = 16                            # rows per partition per DMA chunk
NQ = G // QD                       # 4 DMA chunks per super-tile

F32 = mybir.dt.float32
U32 = mybir.dt.uint32
AF = mybir.ActivationFunctionType


def build(loop_iters: int = 1):
    """Build + compile the per-core program. loop_iters > 1 wraps the body in
    a dynamic loop that reprocesses the same data (timing only)."""
    nc = bacc.Bacc(
        "TRN2", target_bir_lowering=False, debug=False, num_devices=N_CORES
    )
    x = nc.dram_tensor("cost", [ROWS_PER_CORE, D], F32, kind="ExternalInput").ap()
    y = nc.dram_tensor("pred", [ROWS_PER_CORE], F32, kind="ExternalOutput").ap()

    # row = t*TILE_ROWS + p*G + g  ->  partition p holds G consecutive rows,
    # i.e. one contiguous 48 KiB run per partition, DMA'd in 12 KiB chunks.
    x_t = x.rearrange("(t p g) d -> t p (g d)", p=P, g=G)
    y_t = y.rearrange("(t p g) -> t p g", p=P, g=G)

    def body(tc):
        with (
            tc.tile_pool(name="xp", bufs=2) as xp,
            tc.tile_pool(name="sp", bufs=2) as sp,
            tc.tile_pool(name="ep", bufs=2) as ep,
        ):
            for t in range(N_TILES):
                xt = xp.tile([P, G * D], F32)
                for q in range(NQ):
                    c0, c1 = q * QD * D, (q + 1) * QD * D
                    nc.sync.dma_start(xt[:, c0:c1], x_t[t][:, c0:c1])

                v8 = sp.tile([P, G * 8], F32)
                i8 = sp.tile([P, G * 8], U32)
                for g in range(G):
                    nc.vector.max(v8[:, g * 8:(g + 1) * 8],
                                  xt[:, g * D:(g + 1) * D])
                    nc.vector.max_index(i8[:, g * 8:(g + 1) * 8],
                                        v8[:, g * 8:(g + 1) * 8],
                                        xt[:, g * D:(g + 1) * D])

                v8v = v8[:].rearrange("p (g k) -> p g k", k=8)
                i8v = i8[:].rearrange("p (g k) -> p g k", k=8)

                # DVE part of the epilogue has no cross-engine inputs, so the
                # in-order DVE never stalls; the sigmoid-dependent tail runs
                # on the otherwise idle GPSIMD.
                d = ep.tile([P, G], F32)
                nc.vector.tensor_sub(d[:], v8v[:, :, 1], v8v[:, :, 0])
                s = ep.tile([P, G], F32)
                nc.scalar.activation(s[:], d[:], AF.Sigmoid)

                # de-interleave while converting: i2f = [i1f(0:G) | i2f(G:2G)]
                i2f = ep.tile([P, 2 * G], F32)
                nc.vector.tensor_copy(
                    i2f[:].rearrange("p (k g) -> p g k", g=G), i8v[:, :, 0:2]
                )
                u = ep.tile([P, G], F32)
                nc.vector.tensor_sub(u[:], i2f[:, G:2 * G], i2f[:, 0:G])

                w = ep.tile([P, G], F32)
                nc.gpsimd.tensor_mul(w[:], u[:], s[:])
                pt = ep.tile([P, G], F32)
                nc.gpsimd.tensor_add(pt[:], w[:], i2f[:, 0:G])

                nc.sync.dma_start(y_t[t], pt[:])

    with tile.TileContext(nc) as tc:
        if loop_iters == 1:
            body(tc)
        else:
            with tc.For_i(0, loop_iters, 1):
                body(tc)

    nc.compile()
    return nc


_NC_CACHE = {}


def _get_nc(loop_iters: int = 1):
    if loop_iters not in _NC_CACHE:
        _NC_CACHE[loop_iters] = build(loop_iters)
    return _NC_CACHE[loop_iters]


def run(cost: np.ndarray, loop_iters: int = 1) -> np.ndarray:
    nc = _get_nc(loop_iters)
    flat = np.ascontiguousarray(cost.reshape(ROWS, D))
    in_maps = [
        {"cost": flat[c * ROWS_PER_CORE:(c + 1) * ROWS_PER_CORE]}
        for c in range(N_CORES)
    ]
    res = run_bass_kernel_spmd(nc, in_maps, core_ids=list(range(N_CORES)))
    out = np.concatenate(
        [res.results[c]["pred"] for c in range(N_CORES)]
    )
    return out.reshape(B, N).astype(np.float32, copy=False)


def kernel(cost: np.ndarray) -> np.ndarray:
    return run(cost, loop_iters=1)
